# revision 49
# baseline (speedup 1.0000x reference)
"""AntiBiasL1Loss (segment_reduce over 5 grades) on 8 TRN2 NeuronCores.

Algorithm
---------
reference:  seg = round(y_true) in [0,5);  e = |y_pred - y_true|
            sums[g]   = segment_sum(e, seg);  counts[g] = segment_sum(1, seg)
            out = mean_g(sums[g]/counts[g])  over present groups.

Device-side (per core, data-parallel shard of N/8 elements, bf16):
  d   = y_pred - y_true                  (DVE tensor_tensor, into e's slot)
  rp  = 2*relu(d)                        (DVE tensor_scalar dual-op, 4x)
  e   = |d| = rp - d                     (DVE tensor_tensor, in place)
  moving operands: y itself, and w_t = relu(y - t), t = 1..3
                                         (DVE tensor_scalar dual-op, 4x)
  For each [128,128] chunk: 4 accumulating matmuls
      psum_t += e_chunk.T @ moving_t     (TensorE, bf16, PSUM f32)
Because grades are integers, y = sum_t [y>=t] and relu(y-t) =
sum_{u>t} [y>=u], so the four matmul results are W = S1+S2+S3+S4 and
V_t = S_{t+1}+..+S4 where S_t = sum(e*[y>=t]); differences on the host
recover every masked segment sum without computing any 0/1 mask tensor.

Sentinel trick: the host packs the data so that every 128-column chunk has
127 real columns plus one SENTINEL column (y_true=4, y_pred=5), i.e. e=1
for those elements.  In the accumulated [128,128] psum_t:
  diag[n,n], n<127   = masked segment sums (W or V_t per block)
  row  [127, n<127]  = moving-operand column sums  -> counts
  col  [m<127, 127]  = e column sums * sentinel    -> sum(e) (block 3)
so counts and sum(e) come out of the same 4 matmuls -- no accumulator
registers, no extra instructions.  Leftover capacity is padded with zero
columns (y=p=0): they add 0 to every sum and count.

TRN2 constraint driving the structure: EVERY instruction (compute and DMA)
in this walrus build encodes at most ONE semaphore wait ("Too many sync
wait commands" otherwise), and Tile does not legalize multi-waits.  Hence:
one engine (DVE) does all elementwise work, input tiles and e tiles are
single-use SBUF slots (no WAR/WAW waits), each mask op independently
carries its own PSUM-group WAR wait, input DMAs ride the gpsimd SWDGE
queue, and the kernel-tail Drain is stripped to its one load-bearing wait.

Host-side finish: un-telescope sums/counts, per-group means, final mean.
Output is a scalar, so no collective: each core DMAs its 4 [128,128] psum
blocks (one [128,512] f32 tensor) and the host combines 8 of them.
"""

import numpy as np

import concourse.bass as bass
from concourse import mybir, tile
from concourse import tile_sem_assignment as _tsa
from concourse.bass_utils import run_bass_kernel_spmd

# All SWDGE (gpsimd-issued) DMAs share one completion-sem lane: fewer DMA
# procs means fewer waits on the kernel-tail Drain, whose hardware encoding
# also has a small wait-slot budget.
_tsa.NUM_SWDGE_GLOBAL_SEMS = 1

P = 128
CORES = 8
N_TOTAL = 16_777_216
SHARD = N_TOTAL // CORES          # 2_097_152
FREE = SHARD // P                 # 16384 real columns per core
CHUNK = 128                       # matmul chunk cols (127 real + 1 sentinel)
REAL = CHUNK - 1
NCHUNK = -(-FREE // REAL)         # 130 chunks
# Non-uniform tiling (in chunks): small tiles at the start shorten the
# pipeline ramp (PE can't start until the first tile's DVE chain is done),
# small tiles at the end shorten the serial tail.
TILES = (4, 13, 13, 13, 13, 13, 13, 13, 13, 13, 9)
TOTC = NCHUNK * CHUNK             # 16640 packed cols per core
THRESHOLDS = (0.5, 1.5, 2.5, 3.5)
F32 = mybir.dt.float32
BF16 = mybir.dt.bfloat16
assert sum(TILES) == NCHUNK and NCHUNK * REAL >= FREE


def build_kernel(tiles=TILES) -> bass.Bass:
    nc = bass.Bass(target_bir_lowering=False, debug=False)

    # interleaved input: per tile j of c_j chunks (T_j = c_j*CHUNK cols),
    # cols [off, off+T_j) = y_pred, [off+T_j, off+2*T_j) = y_true.  bf16:
    # the device rounds to bf16 anyway, so converting on the host halves
    # DMA traffic and doubles the DVE perf-mode rates.
    totc = sum(tiles) * CHUNK
    x_ext = nc.declare_dram_parameter("xin", [P, 2 * totc], BF16, isOutput=False)
    out_ext = nc.declare_dram_parameter("out", [P, 4 * CHUNK], F32, isOutput=True)

    with tile.TileContext(nc) as tc:
        with (
            # bufs=nt: every input tile gets its own SBUF slot, so input DMAs
            # never carry a WAW wait from slot reuse (DMACopy encodes at most
            # one sync wait).
            tc.tile_pool(name="inp", bufs=len(tiles)) as inp,
            tc.tile_pool(name="epool", bufs=len(tiles)) as epool,
            tc.tile_pool(name="mid", bufs=3) as mid,
            tc.tile_pool(name="stat", bufs=1) as stat,
            tc.tile_pool(name="psum", bufs=1, space=bass.MemorySpace.PSUM) as psum,
        ):
            psum_t = [
                psum.tile([P, CHUNK], F32, tag=f"ps{t}", name=f"ps{t}")
                for t in range(4)
            ]

            off = 0
            for j, cj in enumerate(tiles):
                tile_c = cj * CHUNK
                nch = cj
                xt = inp.tile([P, 2 * tile_c], BF16, tag="xt", name=f"xt{j}")
                nc.gpsimd.dma_start(
                    out=xt[:, :], in_=x_ext[:, off : off + 2 * tile_c]
                )
                off += 2 * tile_c
                pt = xt[:, :tile_c]
                yt = xt[:, tile_c:]

                # d = p - y goes straight into the fresh e slot; |d| is
                # then computed in place (d is never needed afterwards).
                e = epool.tile([P, tile_c], BF16, tag="e")
                nc.vector.tensor_tensor(e[:, :], pt, yt, mybir.AluOpType.subtract)

                # Moving operands for the 4 matmuls: y itself plus
                # w_t = relu(y - t) = sum_{u>t} [y >= u] for t = 1..3 (exact
                # for integer grades).  The telescoped masked sums come out
                # as differences on the host.  Each op encodes at most ONE
                # semaphore wait; every w reads y straight from the DMA tile
                # (RAW already observed via the subtract), so the single
                # wait slot takes the WAR on its slot's PSUM-group readers.
                # |d| = 2*relu(d) - d  (no abs-class ALU op in the real
                # ISA; dual-op tensor_scalar stays on the 4x perf mode).
                # e is finished BEFORE the w ops so the PE can load the
                # stationary and start this tile's y-matmuls while the DVE
                # still produces the remaining moving operands.
                rp = mid.tile([P, tile_c], BF16, tag="rp")
                nc.vector.tensor_scalar(
                    rp[:, :], e[:, :], 0.0, 2.0,
                    mybir.AluOpType.max, op1=mybir.AluOpType.mult,
                )
                nc.vector.tensor_tensor(
                    e[:, :], rp[:, :], e[:, :], mybir.AluOpType.subtract
                )
                masks = [yt]
                for t in (1.0, 2.0, 3.0):
                    m = mid.tile([P, tile_c], BF16, tag=f"m{t}")
                    nc.vector.tensor_scalar(
                        m[:, :], yt, t, 0.0,
                        mybir.AluOpType.subtract, op1=mybir.AluOpType.max,
                    )
                    masks.append(m[:, :])

                for c in range(nch):
                    csl = slice(c * CHUNK, (c + 1) * CHUNK)
                    first = j == 0 and c == 0
                    last = j == len(tiles) - 1 and c == nch - 1
                    for t in range(4):
                        nc.tensor.matmul(
                            psum_t[t][:, :],
                            e[:, csl],
                            masks[t][:, csl],
                            start=first,
                            stop=last,
                        )

            psum_sb = stat.tile([P, 4 * CHUNK], F32, tag="psb", name="psum_sb")
            for t in range(4):
                nc.vector.tensor_copy(
                    psum_sb[:, t * CHUNK : (t + 1) * CHUNK], psum_t[t][:, :]
                )
            nc.sync.dma_start(out=out_ext[:, :], in_=psum_sb[:, :])

    # The kernel-tail Drain waits on every active proc, but its hardware
    # encoding (like every other instruction here) holds only ONE sync wait.
    # All but the final output DMA's completion are transitively implied:
    # engine queues are in-order and the all-engine barrier follows the
    # drain; every input-DMA completion was already awaited by its DVE
    # consumer.  Keep only the DMAHW wait (the output DMA).
    for b in nc.m.functions[0].blocks:
        for i in b.instructions:
            si = i.sync_info
            if type(i).__name__ == "InstDrain" and si and len(si.on_wait) > 1:
                keep = [w for w in si.on_wait if w.ant_name.startswith("DMAHW")]
                assert len(keep) == 1, [w.ant_name for w in si.on_wait]
                i.sync_info = mybir.SyncInfo(
                    on_wait=keep, on_update=list(si.on_update)
                )
    return nc


def combine_outputs(outs, n_total: int = N_TOTAL) -> np.float32:
    """Host-side finish.

    psum block 0 (moving = y):            diag = W  = S1+S2+S3+S4,
                                          row  = Cy = C1+C2+C3+C4
    psum block t (moving = relu(y-t)):    diag = V_t = S_{t+1}+..+S4,
                                          row  = D_t = C_{t+1}+..+C4
    block 3's sentinel column (relu(4-3)=1) gives sum(e).
    S_t = sum(e*[y>=t]), C_t = count(y>=t); un-telescope, then the 5-way
    mean of per-grade means.
    """
    v = np.zeros(4, np.float64)   # W, V1, V2, V3
    c = np.zeros(4, np.float64)   # Cy, D1, D2, D3
    sum_e = 0.0
    for o in outs:
        o = np.asarray(o, np.float64)
        for t in range(4):
            blk = o[:, t * CHUNK : (t + 1) * CHUNK]
            v[t] += np.trace(blk[:REAL, :REAL])
            c[t] += blk[REAL, :REAL].sum()
        sum_e += o[:, 3 * CHUNK : 4 * CHUNK][:REAL, REAL].sum()

    # W - V1 = sum(e*(y - relu(y-1))) = sum(e*[y>=1]) = S_1, etc: the
    # differences ARE the cumulative tail sums S_t directly.
    s_thr = np.array([v[0] - v[1], v[1] - v[2], v[2] - v[3], v[3]])
    c_thr = np.array([c[0] - c[1], c[1] - c[2], c[2] - c[3], c[3]])
    s_cum = np.array([sum_e, *s_thr, 0.0])
    c_cum = np.array([float(n_total), *c_thr, 0.0])
    sums = s_cum[:-1] - s_cum[1:]
    counts = c_cum[:-1] - c_cum[1:]
    present = counts > 0
    means = np.where(present, sums / np.where(present, counts, 1.0), 0.0)
    return np.float32(means.sum() / present.sum())


def pack_inputs(y_pred: np.ndarray, y_true: np.ndarray):
    """[N] f32 x2 -> per-core bf16 [P, 2*TOTC]: sentinel col per chunk,
    zero-col padding, then per-tile y_pred|y_true interleave."""
    import ml_dtypes
    bf16 = np.dtype(ml_dtypes.bfloat16)
    p = np.ascontiguousarray(y_pred, np.float32).reshape(CORES, P, FREE)
    y = np.ascontiguousarray(y_true, np.float32).reshape(CORES, P, FREE)
    pc = np.zeros((CORES, P, NCHUNK, CHUNK), bf16)
    yc = np.zeros((CORES, P, NCHUNK, CHUNK), bf16)
    tmp = np.zeros((CORES, P, NCHUNK * REAL), np.float32)
    tmp[:, :, :FREE] = p
    pc[:, :, :, :REAL] = tmp.reshape(CORES, P, NCHUNK, REAL)
    tmp[:, :, :FREE] = y
    yc[:, :, :, :REAL] = tmp.reshape(CORES, P, NCHUNK, REAL)
    pc[:, :, :, REAL] = 5.0  # sentinel: e = |5-4| = 1
    yc[:, :, :, REAL] = 4.0  # sentinel: stationary ones-column via e
    pc = pc.reshape(CORES, P, TOTC)
    yc = yc.reshape(CORES, P, TOTC)
    x = np.empty((CORES, P, 2 * TOTC), bf16)
    off = 0
    coff = 0
    for cj in TILES:
        t = cj * CHUNK
        x[:, :, off : off + t] = pc[:, :, coff : coff + t]
        x[:, :, off + t : off + 2 * t] = yc[:, :, coff : coff + t]
        off += 2 * t
        coff += t
    return x


def run(y_pred: np.ndarray, y_true: np.ndarray, trace: bool = False, **kw):
    x = pack_inputs(y_pred, y_true)
    in_maps = [{"xin": x[i]} for i in range(CORES)]
    nc = build_kernel()
    res = run_bass_kernel_spmd(
        nc, in_maps, core_ids=list(range(CORES)), trace=trace, **kw
    )
    outs = [res.results[i]["out"] for i in range(CORES)]
    return np.asarray(combine_outputs(outs), np.float32), res


def kernel(y_pred: np.ndarray, y_true: np.ndarray) -> np.ndarray:
    return run(y_pred, y_true)[0]
